# revision 14
# baseline (speedup 1.0000x reference)
"""Exact KNN collision kernel for trn2 (8 NeuronCores) — pruned-candidate version.

Computes nn[b,n] = argmin_m |vertices[b,n] - collider[b, cvi[m]]|^2 with the
reference's exact fp32 arithmetic and first-occurrence tie-breaking.

Host side (cheap, o(rows x U) work):
  - dedup the gathered collider points (U ~ 3090 candidates)
  - per batch: recursive longest-axis median splits give 128 spatially
    compact chunks of 128 query rows; for each chunk a PROVABLY sufficient
    candidate list via half-space domination pruning:
        drop j iff exists k with  d2(z,j) - d2(z,k) - 2*|j-k|*r >= 1e-3
    which implies d2(q,j) > d2(q,k) + 1e-3 for every query q in the chunk
    ball(z,r); 1e-3 dwarfs all fp32 rounding slack (<1e-4), so the reference
    fp32 argmin and ALL its fp32 ties stay in the list.
  - chunks dealt to 8 cores by sorted rank (rank widths uniform across
    cores -> one SPMD program); lists stored by DESCENDING dedup slot so the
    scan's last-max tie-break == reference first-occurrence.

Device side, per slot (bitwise-verified on hw by micro tests):
  - mm1: fp32 K=3 matmul (dot; bitwise equal to the reference einsum)
  - mm2: bf16 K=3 matmul ones x (-c2h in 3 disjoint-mantissa bf16 limbs
    summing EXACTLY to -c2h) accumulated into the same PSUM
    -> psum = fp32(dot - c2h) = -d2/2 bitwise  (MM2_EXACT micro test)
  - one-piece slots: fused DVE argmax scan reads PSUM directly; multi-piece
    slots: ACT drains pieces to SBUF, one scan over the whole row.
Layout: 3 mega-groups at base partitions 0/32/64 (PE constraint).  Each
group's work forms a column STREAM ([verts | cands] per slot); streams are
cut into NCHK uniform [68, T] chunk tiles so the serial DMA (~0.385 ns per
per-partition byte) runs just ahead of the PE.  Processing: 8 smallest
slots first (fast start), then descending width (short tail).
"""
import sys
import numpy as np

_BASS_PATH = "/opt/trn_rl_repo"
if _BASS_PATH not in sys.path:
    sys.path.insert(0, _BASS_PATH)

B, N, V, M = 4, 16384, 6890, 4096
NCORES = 8
NT = 64                  # slots per core
NCHUNK_B = 128           # chunks per batch
KNN = 64
NDOMZ = 384
ABS_EPS = 1e-3
NEG = np.float32(-3.4028235e38)
PAD_LIMB = np.float32(-2.5e29)
NCHK = 12
PIECE = 1024

_PROGRAM_CACHE = {}


def _register_op(name, make_spec):
    from concourse import dve_ops
    from concourse.dve_spec import lower
    from concourse.dve_spec import _has_src1
    from concourse.dve_uop import DveOpSpec

    if name in dve_ops._SUB_OPCODE_FOR_NAME:
        return dve_ops.CUSTOM_DVE_SPECS[name]._antop
    spec = make_spec()
    shas = {}
    for ver in ("v3", "v4"):
        tmp = DveOpSpec(name=name, opcode=31, uops=lower(spec, ver=ver),
                        rd1_en=_has_src1(spec))
        shas[ver] = tmp.sha(ver)
    op = dve_ops.DveOp(name, spec, subdim=False, uops_sha=shas)
    row = max(dve_ops._SUB_OPCODE_FOR_NAME.values()) + 1
    assert row < 0x20
    dve_ops.OPS.append(op)
    dve_ops.CUSTOM_DVE_SPECS[name] = spec
    dve_ops._SUB_OPCODE_FOR_NAME[name] = row
    spec._antop = op
    return op


def _register_argmax_scan1():
    """accum = fp32 index of the LAST element equal to the running max."""
    from concourse.dve_spec import (Spec, Src0, Idx, MaxNeg, maxx, select,
                                    scan, AluOp)

    def make():
        def _ref(in0, *a):
            s2 = np.asarray(in0, np.float32).reshape(np.shape(in0)[0], -1)
            m = np.maximum.accumulate(s2, axis=-1)
            idx = np.broadcast_to(
                np.arange(s2.shape[1], dtype=np.float32), s2.shape)
            body = np.where(s2 >= m, idx, NEG).astype(np.float32)
            acc = body.max(axis=-1, keepdims=True).astype(np.float32)
            return body.reshape(np.shape(in0)), acc

        sm = scan(AluOp.MAX, Src0)
        body = select(Src0 >= sm, Idx, MaxNeg)
        return Spec(body=body, accum=maxx, reference=_ref)

    return _register_op("ARGMAX_SCAN1_ANT", make)


def _trunc16(x):
    return (np.ascontiguousarray(x, np.float32).view(np.uint32)
            & np.uint32(0xFFFF0000)).view(np.float32)


def _split3(x):
    """x == a + b + c exactly; each limb bf16-representable."""
    a = _trunc16(x)
    r = (x - a).astype(np.float32)
    b = _trunc16(r)
    cc = (r - b).astype(np.float32)
    return a, b, cc


def _kd_perm(pts, n_levels):
    idx = np.arange(len(pts))
    stack = [idx]
    for _ in range(n_levels):
        nxt = []
        for g in stack:
            p = pts[g]
            ax = int(np.argmax(p.max(0) - p.min(0)))
            o = np.argsort(p[:, ax], kind="stable")
            h = len(g) // 2
            nxt.append(g[o[:h]])
            nxt.append(g[o[h:]])
        stack = nxt
    return np.concatenate(stack)


def _ceil8(x):
    return max(8, (int(x) + 7) // 8 * 8)


def _plan_layout(widths):
    """Group-stream layout.  Each slot is assigned wholly to the least-loaded
    mega-group; the group's column stream is [verts(128) | cands(W)] per
    slot.  Streams are cut into NCHK tiles of T columns; candidate spans are
    further cut at tile boundaries and PIECE size.  Vertex columns never
    straddle a tile (padded to the boundary instead).

    Returns order, vpl[r]=(k,m,off), ppl[r]=[(k,m,xoff,hoff,a,b)...], T, LH.
    """
    asc = sorted(range(NT), key=lambda r: (widths[r], r))
    order = asc[:8] + list(reversed(asc[8:]))
    total = sum(128 + w for w in widths)
    T = _ceil8(int(total / 3.0 / NCHK) + 160)
    T0 = 296                       # small first tile -> compute starts fast

    def tile_of(pos):
        return 0 if pos < T0 else 1 + (pos - T0) // T

    def off_of(pos):
        return pos if pos < T0 else (pos - T0) % T

    def room_of(pos):
        return (T0 - pos) if pos < T0 else T - (pos - T0) % T

    G = [0, 0, 0]                  # group stream cursors (x cols)
    plan = []                      # (r, m, g0)
    for r in order:
        m = int(np.argmin(G))
        g0 = G[m]
        if room_of(g0) < 128:      # verts must not straddle a tile
            g0 += room_of(g0)
        plan.append((r, m, g0))
        G[m] = g0 + 128 + widths[r]
    nchk = max(tile_of(max(g - 1, 0)) + 1 for g in G)

    vpl = {}
    ppl = {}
    hcur = {}                      # (k, m) -> h cursor
    for r, m, g0 in plan:
        vpl[r] = (tile_of(g0), m, off_of(g0))
        pieces = []
        W = widths[r]
        pos = g0 + 128
        a = 0
        while a < W:
            k = tile_of(pos)
            b = min(a + min(PIECE, room_of(pos)), W)
            ho = hcur.get((k, m), 0)
            pieces.append((k, m, off_of(pos), ho, a, b))
            hcur[(k, m)] = ho + (b - a)
            pos += b - a
            a = b
        ppl[r] = pieces
    LH = [8] * nchk
    for (k, m), h in hcur.items():
        LH[k] = max(LH[k], h)
    TS = [T0] + [T] * (nchk - 1)
    return order, vpl, ppl, TS, LH, nchk


def _build_program(widths, order, vpl, ppl, TS, LH, nchk, wmax):
    import concourse.bacc as bacc
    import concourse.mybir as mybir
    import concourse.tile as tile

    f32 = mybir.dt.float32
    bf16 = mybir.dt.bfloat16

    nc = bacc.Bacc("TRN2", target_bir_lowering=False, debug=False,
                   num_devices=NCORES)
    xds = [nc.dram_tensor(f"x{k}", [68, TS[k]], f32, kind="ExternalInput")
           for k in range(nchk)]
    hds = [nc.dram_tensor(f"h{k}", [68, LH[k]], bf16, kind="ExternalInput")
           for k in range(nchk)]
    outd = nc.dram_tensor("idx", [128, NT], f32, kind="ExternalOutput")

    with tile.TileContext(nc) as tc:
        with (
            tc.tile_pool(name="const", bufs=1) as cpool,
            tc.tile_pool(name="work", bufs=2) as wpool,
            tc.tile_pool(name="psum", bufs=1, space="PSUM") as ppool,
        ):
            x_sb = [cpool.tile([68, TS[k]], f32, tag=f"x{k}", name=f"x{k}")
                    for k in range(nchk)]
            h_sb = [cpool.tile([68, LH[k]], bf16, tag=f"h{k}", name=f"h{k}")
                    for k in range(nchk)]
            ones = cpool.tile([67, 128], bf16)
            h0n = 48
            acc0 = cpool.tile([128, h0n], f32)
            acc1 = cpool.tile([128, NT - 1 - h0n], f32)
            acc2 = cpool.tile([128, 1], f32)
            for k in range(nchk):
                nc.sync.dma_start(x_sb[k][:], xds[k][:])
                nc.sync.dma_start(h_sb[k][:], hds[k][:])
            nc.gpsimd.memset(ones[:], 1.0)

            op = _register_argmax_scan1()

            pi = 0
            for si, r in enumerate(order):
                W = widths[r]
                kv, mv, xov = vpl[r]
                bpv = 32 * mv
                lhsT = x_sb[kv][bpv:bpv + 3, xov:xov + 128]
                if si < h0n:
                    acct = acc0[:, si:si + 1]
                elif si < NT - 1:
                    acct = acc1[:, si - h0n:si - h0n + 1]
                else:
                    acct = acc2[:, 0:1]
                multi = len(ppl[r]) > 1
                dcp = None
                if multi:
                    dcp = wpool.tile([128, wmax], f32, tag="dcp", name="dcp")
                scr = wpool.tile([128, wmax], f32, tag="scr", name="scr")
                last_pt = None
                for (k, m, xo, ho, a, b) in ppl[r]:
                    bp = 32 * m
                    pt = ppool.tile([128, b - a], f32, tag=f"ps{pi % 4}")
                    pi += 1
                    for a2 in range(a, b, 512):
                        b2 = min(a2 + 512, b)
                        nc.tensor.matmul(
                            pt[:, a2 - a:b2 - a], lhsT,
                            x_sb[k][bp:bp + 3, xo + (a2 - a):xo + (b2 - a)],
                            start=True, stop=False)
                        nc.tensor.matmul(
                            pt[:, a2 - a:b2 - a], ones[bp:bp + 3, :],
                            h_sb[k][bp:bp + 3, ho + (a2 - a):ho + (b2 - a)],
                            start=False, stop=True)
                    if multi:
                        nc.scalar.copy(dcp[:, a:b], pt[:])
                    else:
                        last_pt = pt
                src = dcp if multi else last_pt
                nc.vector._custom_dve(op, out=scr[:, :W], in0=src[:, :W],
                                      accum_out=acct)
                if si == h0n - 1:
                    nc.sync.dma_start(outd[:, :h0n], acc0[:])
                if si == NT - 2:
                    nc.sync.dma_start(outd[:, h0n:NT - 1], acc1[:])
            nc.sync.dma_start(outd[:, NT - 1:], acc2[:])
    nc.compile()
    return nc


def _get_program(widths, order, vpl, ppl, TS, LH, nchk, wmax):
    key = (tuple(widths), tuple(TS), tuple(LH))
    if key not in _PROGRAM_CACHE:
        _PROGRAM_CACHE[key] = _build_program(widths, order, vpl, ppl, TS, LH,
                                             nchk, wmax)
    return _PROGRAM_CACHE[key]


def kernel(vertices, collider, collision_vertices, _want_trace=False):
    from concourse.bass_utils import run_bass_kernel_spmd
    import ml_dtypes

    v = np.ascontiguousarray(np.asarray(vertices), dtype=np.float32)
    c = np.ascontiguousarray(np.asarray(collider), dtype=np.float32)
    cvi = np.asarray(collision_vertices).astype(np.int64)

    u, first_pos = np.unique(cvi, return_index=True)
    order0 = np.argsort(first_pos)
    u = u[order0]
    first_pos = first_pos[order0].astype(np.int32)
    U = len(u)

    # ---- chunk lists (fp32 host geometry; margins dwarf fp32 error) --------
    chunks = []
    for b in range(B):
        cv64 = c[b][u].astype(np.float64)
        d2cc = ((cv64[:, None] - cv64[None]) ** 2).sum(-1).astype(np.float32)
        np.fill_diagonal(d2cc, np.inf)
        nnidx = np.argpartition(d2cc, KNN, axis=1)[:, :KNN]
        dknn = 2.0 * np.sqrt(d2cc[np.arange(U)[:, None], nnidx])
        q = v[b]
        perm = _kd_perm(q, 7)
        cv32 = cv64.astype(np.float32)
        for t in range(NCHUNK_B):
            rows = perm[t * 128:(t + 1) * 128]
            pts = q[rows].astype(np.float64)
            z = pts.mean(0)
            r = np.float32(np.sqrt(((pts - z) ** 2).sum(1).max()))
            d2z = ((cv64 - z) ** 2).sum(1).astype(np.float32)
            domz = np.argpartition(d2z, NDOMZ)[:NDOMZ]
            ddz = 2.0 * np.sqrt(
                ((cv32[:, None] - cv32[domz][None]) ** 2).sum(-1))
            dominated = ((d2z[:, None] - d2z[domz][None]) - ddz * r
                         >= ABS_EPS).any(1)
            dominated |= ((d2z[:, None] - d2z[nnidx]) - dknn * r
                          >= ABS_EPS).any(1)
            keep = np.nonzero(~dominated)[0]
            keep_desc = keep[::-1].copy()
            chunks.append((b, rows, keep_desc))

    # ---- rank-deal chunks to cores (tight uniform rank widths) -------------
    wid = np.array([_ceil8(len(ch[2])) for ch in chunks])
    aorder = np.argsort(-wid, kind="stable")
    assign = [[0] * NT for _ in range(NCORES)]
    widths = [0] * NT
    for r in range(NT):
        blk = aorder[r * NCORES:(r + 1) * NCORES]
        widths[r] = int(wid[blk].max())
        for cc in range(NCORES):
            assign[cc][r] = int(blk[cc])
    wmax = max(widths)

    sorder, vpl, ppl, TS, LH, nchk = _plan_layout(widths)

    # ---- pack per-core inputs ---------------------------------------------
    c2h_all = []
    for b in range(B):
        cvb = c[b][u]
        c2h_all.append(((cvb * cvb).sum(-1, dtype=np.float32)
                        * np.float32(0.5)).astype(np.float32))

    in_maps = []
    for core in range(NCORES):
        xs = [np.zeros((68, TS[k]), np.float32) for k in range(nchk)]
        hs = [np.full((68, LH[k]), PAD_LIMB, np.float32) for k in range(nchk)]
        for r in range(NT):
            b, rows, keep_desc = chunks[assign[core][r]]
            L = len(keep_desc)
            W = widths[r]
            kv, mv, xov = vpl[r]
            xs[kv][32 * mv:32 * mv + 3, xov:xov + 128] = v[b][rows].T
            coords = np.zeros((3, W), np.float32)
            coords[:, :L] = c[b][u[keep_desc]].T
            la, lb, lc = _split3(-c2h_all[b][keep_desc])
            limbs = np.full((3, W), PAD_LIMB, np.float32)
            limbs[0, :L] = la
            limbs[1, :L] = lb
            limbs[2, :L] = lc
            for (k, m, xo, ho, a, bnd) in ppl[r]:
                bp = 32 * m
                xs[k][bp:bp + 3, xo:xo + (bnd - a)] = coords[:, a:bnd]
                hs[k][bp:bp + 3, ho:ho + (bnd - a)] = limbs[:, a:bnd]
        im = {}
        for k in range(nchk):
            im[f"x{k}"] = np.ascontiguousarray(xs[k])
            im[f"h{k}"] = np.ascontiguousarray(
                hs[k].astype(ml_dtypes.bfloat16))
        in_maps.append(im)

    nc = _get_program(widths, sorder, vpl, ppl, TS, LH, nchk, wmax)
    res = run_bass_kernel_spmd(nc, in_maps, core_ids=list(range(NCORES)))

    # ---- unpack ------------------------------------------------------------
    nn = np.zeros((B, N), np.int32)
    for core in range(NCORES):
        kk = np.rint(res.results[core]["idx"]).astype(np.int64)   # [128, NT]
        for si, r in enumerate(sorder):
            b, rows, keep_desc = chunks[assign[core][r]]
            nn[b, rows] = first_pos[keep_desc[kk[:, si]]]
    batch_idx = np.broadcast_to(np.arange(B, dtype=np.int32)[:, None], nn.shape)
    outv = np.stack([batch_idx, nn], axis=-1).astype(np.int32)
    if _want_trace:
        return outv, (res, in_maps)
    return outv


# revision 15
# speedup vs baseline: 1.0042x; 1.0042x over previous
"""Exact KNN collision kernel for trn2 (8 NeuronCores) — pruned-candidate version.

Computes nn[b,n] = argmin_m |vertices[b,n] - collider[b, cvi[m]]|^2 with the
reference's exact fp32 arithmetic and first-occurrence tie-breaking.

Host side (cheap, o(rows x U) work):
  - dedup the gathered collider points (U ~ 3090 candidates)
  - per batch: recursive longest-axis median splits give 128 spatially
    compact chunks of 128 query rows; for each chunk a PROVABLY sufficient
    candidate list via half-space domination pruning:
        drop j iff exists k with  d2(z,j) - d2(z,k) - 2*|j-k|*r >= 1e-3
    which implies d2(q,j) > d2(q,k) + 1e-3 for every query q in the chunk
    ball(z,r); 1e-3 dwarfs all fp32 rounding slack (<1e-4), so the reference
    fp32 argmin and ALL its fp32 ties stay in the list.
  - chunks dealt to 8 cores by sorted rank (rank widths uniform across
    cores -> one SPMD program); lists stored by DESCENDING dedup slot so the
    scan's last-max tie-break == reference first-occurrence.

Device side, per slot (bitwise-verified on hw by micro tests):
  - mm1: fp32 K=3 matmul (dot; bitwise equal to the reference einsum)
  - mm2: bf16 K=3 matmul ones x (-c2h in 3 disjoint-mantissa bf16 limbs
    summing EXACTLY to -c2h) accumulated into the same PSUM
    -> psum = fp32(dot - c2h) = -d2/2 bitwise  (MM2_EXACT micro test)
  - one-piece slots: fused DVE argmax scan reads PSUM directly; multi-piece
    slots: ACT drains pieces to SBUF, one scan over the whole row.
Layout: 3 mega-groups at base partitions 0/32/64 (PE constraint).  Each
group's work forms a column STREAM ([verts | cands] per slot); streams are
cut into NCHK uniform [68, T] chunk tiles so the serial DMA (~0.385 ns per
per-partition byte) runs just ahead of the PE.  Processing: 8 smallest
slots first (fast start), then descending width (short tail).
"""
import sys
import numpy as np

_BASS_PATH = "/opt/trn_rl_repo"
if _BASS_PATH not in sys.path:
    sys.path.insert(0, _BASS_PATH)

B, N, V, M = 4, 16384, 6890, 4096
NCORES = 8
NT = 64                  # slots per core
NCHUNK_B = 128           # chunks per batch
KNN = 64
NDOMZ = 384
ABS_EPS = 1e-3
NEG = np.float32(-3.4028235e38)
PAD_LIMB = np.float32(-2.5e29)
NCHK = 12
PIECE = 1024

_PROGRAM_CACHE = {}


def _register_op(name, make_spec):
    from concourse import dve_ops
    from concourse.dve_spec import lower
    from concourse.dve_spec import _has_src1
    from concourse.dve_uop import DveOpSpec

    if name in dve_ops._SUB_OPCODE_FOR_NAME:
        return dve_ops.CUSTOM_DVE_SPECS[name]._antop
    spec = make_spec()
    shas = {}
    for ver in ("v3", "v4"):
        tmp = DveOpSpec(name=name, opcode=31, uops=lower(spec, ver=ver),
                        rd1_en=_has_src1(spec))
        shas[ver] = tmp.sha(ver)
    op = dve_ops.DveOp(name, spec, subdim=False, uops_sha=shas)
    row = max(dve_ops._SUB_OPCODE_FOR_NAME.values()) + 1
    assert row < 0x20
    dve_ops.OPS.append(op)
    dve_ops.CUSTOM_DVE_SPECS[name] = spec
    dve_ops._SUB_OPCODE_FOR_NAME[name] = row
    spec._antop = op
    return op


def _register_argmax_scan1():
    """accum = fp32 index of the LAST element equal to the running max."""
    from concourse.dve_spec import (Spec, Src0, Idx, MaxNeg, maxx, select,
                                    scan, AluOp)

    def make():
        def _ref(in0, *a):
            s2 = np.asarray(in0, np.float32).reshape(np.shape(in0)[0], -1)
            m = np.maximum.accumulate(s2, axis=-1)
            idx = np.broadcast_to(
                np.arange(s2.shape[1], dtype=np.float32), s2.shape)
            body = np.where(s2 >= m, idx, NEG).astype(np.float32)
            acc = body.max(axis=-1, keepdims=True).astype(np.float32)
            return body.reshape(np.shape(in0)), acc

        sm = scan(AluOp.MAX, Src0)
        body = select(Src0 >= sm, Idx, MaxNeg)
        return Spec(body=body, accum=maxx, reference=_ref)

    return _register_op("ARGMAX_SCAN1_ANT", make)


def _trunc16(x):
    return (np.ascontiguousarray(x, np.float32).view(np.uint32)
            & np.uint32(0xFFFF0000)).view(np.float32)


def _split3(x):
    """x == a + b + c exactly; each limb bf16-representable."""
    a = _trunc16(x)
    r = (x - a).astype(np.float32)
    b = _trunc16(r)
    cc = (r - b).astype(np.float32)
    return a, b, cc


def _kd_perm(pts, n_levels):
    idx = np.arange(len(pts))
    stack = [idx]
    for _ in range(n_levels):
        nxt = []
        for g in stack:
            p = pts[g]
            ax = int(np.argmax(p.max(0) - p.min(0)))
            o = np.argsort(p[:, ax], kind="stable")
            h = len(g) // 2
            nxt.append(g[o[:h]])
            nxt.append(g[o[h:]])
        stack = nxt
    return np.concatenate(stack)


def _ceil8(x):
    return max(8, (int(x) + 7) // 8 * 8)


def _plan_layout(widths):
    """Group-stream layout.  Each slot is assigned wholly to the least-loaded
    mega-group; the group's column stream is [verts(128) | cands(W)] per
    slot.  Streams are cut into NCHK tiles of T columns; candidate spans are
    further cut at tile boundaries and PIECE size.  Vertex columns never
    straddle a tile (padded to the boundary instead).

    Returns order, vpl[r]=(k,m,off), ppl[r]=[(k,m,xoff,hoff,a,b)...], T, LH.
    """
    asc = sorted(range(NT), key=lambda r: (widths[r], r))
    order = asc[:8] + list(reversed(asc[8:]))
    total = sum(128 + w for w in widths)
    T = _ceil8(int(total / 3.0 / NCHK) + 160)

    G = [0, 0, 0]                  # group stream cursors (x cols)
    plan = []                      # (r, m, g0)
    for r in order:
        m = int(np.argmin(G))
        g0 = G[m]
        if g0 % T + 128 > T:       # verts must not straddle a tile
            g0 = (g0 // T + 1) * T
        plan.append((r, m, g0))
        G[m] = g0 + 128 + widths[r]
    nchk = max((g + T - 1) // T for g in G)

    vpl = {}
    ppl = {}
    hcur = {}                      # (k, m) -> h cursor
    for r, m, g0 in plan:
        k0 = g0 // T
        vpl[r] = (k0, m, g0 % T)
        pieces = []
        W = widths[r]
        pos = g0 + 128
        a = 0
        while a < W:
            k = pos // T
            room = T - pos % T
            b = min(a + min(PIECE, room), W)
            ho = hcur.get((k, m), 0)
            pieces.append((k, m, pos % T, ho, a, b))
            hcur[(k, m)] = ho + (b - a)
            pos += b - a
            a = b
        ppl[r] = pieces
    LH = [8] * nchk
    for (k, m), h in hcur.items():
        LH[k] = max(LH[k], h)
    return order, vpl, ppl, T, LH, nchk


def _build_program(widths, order, vpl, ppl, T, LH, nchk, wmax):
    import concourse.bacc as bacc
    import concourse.mybir as mybir
    import concourse.tile as tile

    f32 = mybir.dt.float32
    bf16 = mybir.dt.bfloat16

    nc = bacc.Bacc("TRN2", target_bir_lowering=False, debug=False,
                   num_devices=NCORES)
    xds = [nc.dram_tensor(f"x{k}", [68, T], f32, kind="ExternalInput")
           for k in range(nchk)]
    hds = [nc.dram_tensor(f"h{k}", [68, LH[k]], bf16, kind="ExternalInput")
           for k in range(nchk)]
    outd = nc.dram_tensor("idx", [128, NT], f32, kind="ExternalOutput")

    with tile.TileContext(nc) as tc:
        with (
            tc.tile_pool(name="const", bufs=1) as cpool,
            tc.tile_pool(name="work", bufs=2) as wpool,
            tc.tile_pool(name="psum", bufs=1, space="PSUM") as ppool,
        ):
            x_sb = [cpool.tile([68, T], f32, tag=f"x{k}", name=f"x{k}")
                    for k in range(nchk)]
            h_sb = [cpool.tile([68, LH[k]], bf16, tag=f"h{k}", name=f"h{k}")
                    for k in range(nchk)]
            ones = cpool.tile([67, 128], bf16)
            h0n = NT // 2
            acc0 = cpool.tile([128, h0n], f32)
            acc1 = cpool.tile([128, NT - h0n], f32)
            for k in range(nchk):
                nc.sync.dma_start(x_sb[k][:], xds[k][:])
                nc.sync.dma_start(h_sb[k][:], hds[k][:])
            nc.gpsimd.memset(ones[:], 1.0)

            op = _register_argmax_scan1()

            pi = 0
            for si, r in enumerate(order):
                W = widths[r]
                kv, mv, xov = vpl[r]
                bpv = 32 * mv
                lhsT = x_sb[kv][bpv:bpv + 3, xov:xov + 128]
                acct = (acc0[:, si:si + 1] if si < h0n
                        else acc1[:, si - h0n:si - h0n + 1])
                multi = len(ppl[r]) > 1
                dcp = None
                if multi:
                    dcp = wpool.tile([128, wmax], f32, tag="dcp", name="dcp")
                scr = wpool.tile([128, wmax], f32, tag="scr", name="scr")
                last_pt = None
                for (k, m, xo, ho, a, b) in ppl[r]:
                    bp = 32 * m
                    pt = ppool.tile([128, b - a], f32, tag=f"ps{pi % 4}")
                    pi += 1
                    for a2 in range(a, b, 512):
                        b2 = min(a2 + 512, b)
                        nc.tensor.matmul(
                            pt[:, a2 - a:b2 - a], lhsT,
                            x_sb[k][bp:bp + 3, xo + (a2 - a):xo + (b2 - a)],
                            start=True, stop=False)
                        nc.tensor.matmul(
                            pt[:, a2 - a:b2 - a], ones[bp:bp + 3, :],
                            h_sb[k][bp:bp + 3, ho + (a2 - a):ho + (b2 - a)],
                            start=False, stop=True)
                    if multi:
                        nc.scalar.copy(dcp[:, a:b], pt[:])
                    else:
                        last_pt = pt
                src = dcp if multi else last_pt
                nc.vector._custom_dve(op, out=scr[:, :W], in0=src[:, :W],
                                      accum_out=acct)
                if si == h0n - 1:
                    nc.sync.dma_start(outd[:, :h0n], acc0[:])
            nc.sync.dma_start(outd[:, h0n:], acc1[:])
    nc.compile()
    return nc


def _get_program(widths, order, vpl, ppl, T, LH, nchk, wmax):
    key = (tuple(widths), T, tuple(LH))
    if key not in _PROGRAM_CACHE:
        _PROGRAM_CACHE[key] = _build_program(widths, order, vpl, ppl, T, LH,
                                             nchk, wmax)
    return _PROGRAM_CACHE[key]


def kernel(vertices, collider, collision_vertices, _want_trace=False):
    from concourse.bass_utils import run_bass_kernel_spmd
    import ml_dtypes

    v = np.ascontiguousarray(np.asarray(vertices), dtype=np.float32)
    c = np.ascontiguousarray(np.asarray(collider), dtype=np.float32)
    cvi = np.asarray(collision_vertices).astype(np.int64)

    u, first_pos = np.unique(cvi, return_index=True)
    order0 = np.argsort(first_pos)
    u = u[order0]
    first_pos = first_pos[order0].astype(np.int32)
    U = len(u)

    # ---- chunk lists (fp32 host geometry; margins dwarf fp32 error) --------
    chunks = []
    for b in range(B):
        cv64 = c[b][u].astype(np.float64)
        d2cc = ((cv64[:, None] - cv64[None]) ** 2).sum(-1).astype(np.float32)
        np.fill_diagonal(d2cc, np.inf)
        nnidx = np.argpartition(d2cc, KNN, axis=1)[:, :KNN]
        dknn = 2.0 * np.sqrt(d2cc[np.arange(U)[:, None], nnidx])
        q = v[b]
        perm = _kd_perm(q, 7)
        cv32 = cv64.astype(np.float32)
        for t in range(NCHUNK_B):
            rows = perm[t * 128:(t + 1) * 128]
            pts = q[rows].astype(np.float64)
            z = pts.mean(0)
            r = np.float32(np.sqrt(((pts - z) ** 2).sum(1).max()))
            d2z = ((cv64 - z) ** 2).sum(1).astype(np.float32)
            domz = np.argpartition(d2z, NDOMZ)[:NDOMZ]
            ddz = 2.0 * np.sqrt(
                ((cv32[:, None] - cv32[domz][None]) ** 2).sum(-1))
            dominated = ((d2z[:, None] - d2z[domz][None]) - ddz * r
                         >= ABS_EPS).any(1)
            dominated |= ((d2z[:, None] - d2z[nnidx]) - dknn * r
                          >= ABS_EPS).any(1)
            keep = np.nonzero(~dominated)[0]
            keep_desc = keep[::-1].copy()
            chunks.append((b, rows, keep_desc))

    # ---- rank-deal chunks to cores (tight uniform rank widths) -------------
    wid = np.array([_ceil8(len(ch[2])) for ch in chunks])
    aorder = np.argsort(-wid, kind="stable")
    assign = [[0] * NT for _ in range(NCORES)]
    widths = [0] * NT
    for r in range(NT):
        blk = aorder[r * NCORES:(r + 1) * NCORES]
        widths[r] = int(wid[blk].max())
        for cc in range(NCORES):
            assign[cc][r] = int(blk[cc])
    wmax = max(widths)

    sorder, vpl, ppl, T, LH, nchk = _plan_layout(widths)

    # ---- pack per-core inputs ---------------------------------------------
    c2h_all = []
    for b in range(B):
        cvb = c[b][u]
        c2h_all.append(((cvb * cvb).sum(-1, dtype=np.float32)
                        * np.float32(0.5)).astype(np.float32))

    in_maps = []
    for core in range(NCORES):
        xs = [np.zeros((68, T), np.float32) for _ in range(nchk)]
        hs = [np.full((68, LH[k]), PAD_LIMB, np.float32) for k in range(nchk)]
        for r in range(NT):
            b, rows, keep_desc = chunks[assign[core][r]]
            L = len(keep_desc)
            W = widths[r]
            kv, mv, xov = vpl[r]
            xs[kv][32 * mv:32 * mv + 3, xov:xov + 128] = v[b][rows].T
            coords = np.zeros((3, W), np.float32)
            coords[:, :L] = c[b][u[keep_desc]].T
            la, lb, lc = _split3(-c2h_all[b][keep_desc])
            limbs = np.full((3, W), PAD_LIMB, np.float32)
            limbs[0, :L] = la
            limbs[1, :L] = lb
            limbs[2, :L] = lc
            for (k, m, xo, ho, a, bnd) in ppl[r]:
                bp = 32 * m
                xs[k][bp:bp + 3, xo:xo + (bnd - a)] = coords[:, a:bnd]
                hs[k][bp:bp + 3, ho:ho + (bnd - a)] = limbs[:, a:bnd]
        im = {}
        for k in range(nchk):
            im[f"x{k}"] = np.ascontiguousarray(xs[k])
            im[f"h{k}"] = np.ascontiguousarray(
                hs[k].astype(ml_dtypes.bfloat16))
        in_maps.append(im)

    nc = _get_program(widths, sorder, vpl, ppl, T, LH, nchk, wmax)
    res = run_bass_kernel_spmd(nc, in_maps, core_ids=list(range(NCORES)))

    # ---- unpack ------------------------------------------------------------
    nn = np.zeros((B, N), np.int32)
    for core in range(NCORES):
        kk = np.rint(res.results[core]["idx"]).astype(np.int64)   # [128, NT]
        for si, r in enumerate(sorder):
            b, rows, keep_desc = chunks[assign[core][r]]
            nn[b, rows] = first_pos[keep_desc[kk[:, si]]]
    batch_idx = np.broadcast_to(np.arange(B, dtype=np.int32)[:, None], nn.shape)
    outv = np.stack([batch_idx, nn], axis=-1).astype(np.int32)
    if _want_trace:
        return outv, (res, in_maps)
    return outv
